# revision 6
# baseline (speedup 1.0000x reference)
"""FFTConv2d kernel for trn2, 8 NeuronCores.

Math: reference einsum 'bchw,oihw->bohw' factorizes:
  Y[b,o] = conv_full(sum_c x[b,c], sum_i w[o,i])[1:-1,1:-1] + bias[o]
i.e. a single-channel 3x3 "same" convolution (flipped kernel) per (b,o).

v3: fp16 end-to-end (PSUM fp32), uneven row-slices [8,40,40,40] for a fast
pipeline start. Per core (2 batches), per slice:
  1. DMA x slice in as fp16, partitions=(b,c), SH+2 row-slots with memset
     zero-rows at the image edges.
  2. Channel-sum matmul (ones lhsT), 4x col-tiled via tile_position
     (0,32g): phase g covers SH/4 output rows; psum partitions 32g+(b,i)
     so the psum->staging copy is [128, 512]-shaped (1 per 4-slot chunk).
  3. Copy PSUM -> phase-split staging fp16 (row stride 130, pad cols
     memset once per buffer).
  4. Flatten: 6 DMAs (b,v) merge the 4 phases into flat P3v [6, ...],
     applying the horizontal shift (2-v) on the src side; overlapping
     halo-slot writes carry identical values.
  5. Taps: 3 DMAs (u) build P9 [19, SH*130] from contiguous P3v windows
     at row offset (2-u)*130; P9 partition = u + 3v + 9b (stride-3 runs).
     Partition 18 holds ones (bias row).
  6. Conv: per <=3-row chunk one fp16 matmul wb[19,128].T @ P9 window ->
     PSUM [128,3,130]; dep-free filler matmuls keep the PE HAM warm.
  7. Copy PSUM -> yt fp16 (drop pad cols), DMA yt -> HBM; host upcasts.
Conv trails channel-sum by 2 slices (DEPTH=2).
"""

import os
import sys
from functools import lru_cache

import numpy as np

for _p in ("/opt/trn_rl_repo", "/root/.axon_site/_ro/trn_rl_repo"):
    if os.path.isdir(_p) and _p not in sys.path:
        sys.path.insert(0, _p)

import ml_dtypes

B, CIN, COUT, H, W = 16, 64, 64, 128, 128
N_CORES = 8
BPC = B // N_CORES  # batches per core = 2
WROW = W + 2  # padded row stride = 130
NPH = 4  # col-tile phases per slice
NPART = BPC * CIN  # 128 input partitions (b, c)
NOUT = BPC * COUT  # 128 output partitions (b, o)
KCONV = BPC * 9 + 1  # 19 conv contraction rows

SLICE_SH = [8, 40, 40, 40]  # output rows per slice (each % 4 == 0)
NS = len(SLICE_SH)
SH_MAX = max(SLICE_SH)
PSLOT_MAX = SH_MAX // NPH + 2  # 12
PHLEN = PSLOT_MAX * WROW + 2  # staging cols = 1562
ST2 = (SH_MAX + 2) * WROW + 2  # P3v cols = 5462
PWIN_MAX = SH_MAX * WROW  # 5200

_SLICE_R0 = np.cumsum([0] + SLICE_SH).tolist()  # row starts
_SLICE_ROWS = []  # clipped input rows
_SLICE_OFF = []
_off = 0
for _s in range(NS):
    _h0 = max(0, _SLICE_R0[_s] - 1)
    _he = min(H, _SLICE_R0[_s + 1] + 1)
    _SLICE_ROWS.append((_h0, _he))
    _SLICE_OFF.append(_off)
    _off += (_he - _h0) * W
XPACK_LEN = _off


@lru_cache(maxsize=1)
def _build():
    import concourse.bacc as bacc
    import concourse.mybir as mybir
    import concourse.tile as tile
    from concourse.ap import AP

    f32 = mybir.dt.float32
    f16 = mybir.dt.float16

    nc = bacc.Bacc("TRN2", target_bir_lowering=False, debug=False, num_devices=N_CORES)

    xp = nc.dram_tensor("xpack", [NPART, XPACK_LEN], f16, kind="ExternalInput")
    ones_cs = nc.dram_tensor("ones_cs", [NPART, BPC * 3], f16, kind="ExternalInput")
    wb = nc.dram_tensor("wb", [KCONV, NOUT], f16, kind="ExternalInput")
    ones_p = nc.dram_tensor("ones_p", [1, PWIN_MAX], f16, kind="ExternalInput")
    y = nc.dram_tensor("y", [NOUT, H * W], f16, kind="ExternalOutput")

    with tile.TileContext(nc) as tc:
        with (
            tc.tile_pool(name="xin", bufs=4) as xin_pool,
            tc.tile_pool(name="sp", bufs=1) as sp_pool,
            tc.tile_pool(name="p3", bufs=1) as p3_pool,
            tc.tile_pool(name="pbuf", bufs=1) as p_pool,
            tc.tile_pool(name="yout", bufs=2) as y_pool,
            tc.tile_pool(name="consts", bufs=1) as c_pool,
            tc.tile_pool(name="cs_ps", bufs=1, space="PSUM") as cs_psum,
            tc.tile_pool(name="cv_ps", bufs=4, space="PSUM") as cv_psum,
            tc.tile_pool(name="wm_ps", bufs=1, space="PSUM") as wm_psum,
        ):
            # consts first on scalar queue, then xin DMAs follow
            ones_t = c_pool.tile([NPART, BPC * 3], f16, tag="ones_cs")
            nc.scalar.dma_start(out=ones_t[:, :], in_=ones_cs.ap()[:, :])
            wb_t = c_pool.tile([KCONV, NOUT], f16, tag="wb")
            nc.scalar.dma_start(out=wb_t[:, :], in_=wb.ap()[:, :])

            NBUF = 2
            NBUF9 = 3
            spbufs = []
            p3bufs = []
            p9bufs = []
            for pi in range(NBUF):
                sp = sp_pool.tile([NPART, PHLEN], f16, tag=f"SP{pi}")
                spt = sp.tensor
                nc.vector.memset(sp[:, 0:1], 0.0)
                nc.vector.memset(
                    AP(tensor=spt, offset=WROW - 1,
                       ap=[[PHLEN, NPART], [WROW, PSLOT_MAX], [1, 2]]),
                    0.0,
                )
                nc.vector.memset(sp[:, PHLEN - 1 : PHLEN], 0.0)
                spbufs.append(sp)
                p3 = p3_pool.tile([BPC * 3, ST2], f16, tag=f"PV{pi}")
                p3bufs.append(p3)
            for pi in range(NBUF9):
                p9 = p_pool.tile([KCONV, PWIN_MAX], f16, tag=f"P9{pi}")
                nc.sync.dma_start(
                    out=p9[KCONV - 1 : KCONV, :], in_=ones_p.ap()[0:1, :]
                )
                p9bufs.append(p9)

            def emit_in(s):
                h0, he = _SLICE_ROWS[s]
                ncols = (he - h0) * W
                nslot = SLICE_SH[s] + 2
                xin = xin_pool.tile([NPART, nslot * W], f16, tag="xin")
                o = _SLICE_OFF[s]
                d0 = (h0 - (_SLICE_R0[s] - 1)) * W  # W for s=0 else 0
                if s == 0:
                    nc.vector.memset(xin[:, 0:W], 0.0)
                if s == NS - 1:
                    nc.vector.memset(xin[:, d0 + ncols :], 0.0)
                nc.scalar.dma_start(
                    out=xin[:, d0 : d0 + ncols], in_=xp.ap()[:, o : o + ncols]
                )
                return xin

            def emit_warm(n):
                # dep-free matmuls that keep the PE HAM clock-gate warm
                # while a P9 chain is in flight.
                for _ in range(n):
                    ps = wm_psum.tile([NOUT, W], f32, tag="warm")
                    nc.tensor.matmul(
                        ps[:, :], wb_t[:, :], wb_t[:, :], start=True, stop=True
                    )

            def emit_cs_and_p(s, xin):
                sh = SLICE_SH[s]
                prows = sh // NPH
                pslot = prows + 2
                nchunk = (pslot + 3) // 4
                sp = spbufs[s % NBUF]
                spt = sp.tensor
                p3 = p3bufs[s % NBUF]
                p3t = p3.tensor
                p9 = p9bufs[s % NBUF9]
                p9t = p9.tensor

                pss = []
                for cb in range(nchunk):
                    sl0 = 4 * cb
                    nrows = min(4, pslot - sl0)
                    n = nrows * W
                    ps = cs_psum.tile([NPART, 4, W], f32, tag=f"cs{cb}")
                    pss.append((ps, sl0, nrows))
                    for g in range(NPH):
                        c0 = (g * prows + sl0) * W
                        nc.tensor.matmul(
                            ps[32 * g : 32 * g + BPC * 3, :nrows, :],
                            ones_t[:, :],
                            xin[:, c0 : c0 + n],
                            start=True,
                            stop=True,
                            tile_position=(0, 32 * g),
                        )
                for cb, (ps, sl0, nrows) in enumerate(pss):
                    dst = AP(
                        tensor=spt,
                        offset=sl0 * WROW + 1,
                        ap=[[PHLEN, NPART], [WROW, nrows], [1, W]],
                    )
                    src = ps[:, :nrows, :]
                    if cb == 1:
                        nc.vector.tensor_copy(dst, src)
                    else:
                        nc.scalar.copy(dst, src)

                # flatten: phase-split staging -> flat P3v with (2-v) shift
                for b in range(BPC):
                    for v in range(3):
                        p = 3 * b + v
                        nc.gpsimd.dma_start(
                            out=AP(
                                tensor=p3t,
                                offset=p * ST2,
                                ap=[[ST2, 1], [prows * WROW, NPH],
                                    [1, pslot * WROW]],
                            ),
                            in_=AP(
                                tensor=spt,
                                offset=p * PHLEN + (2 - v),
                                ap=[[32 * PHLEN, NPH], [1, pslot * WROW]],
                            ),
                        )
                # taps: P3v windows -> P9 (partition u + 3v + 9b)
                pwin = sh * WROW
                for u in range(3):
                    nc.sync.dma_start(
                        out=AP(
                            tensor=p9t,
                            offset=u * PWIN_MAX,
                            ap=[[3 * PWIN_MAX, 6], [1, pwin]],
                        ),
                        in_=AP(
                            tensor=p3t,
                            offset=(2 - u) * WROW,
                            ap=[[ST2, 6], [1, pwin]],
                        ),
                        single_packet=True,
                    )
                return p9

            def emit_cv_and_out(s, p9):
                sh = SLICE_SH[s]
                r0 = _SLICE_R0[s]
                yt = y_pool.tile([NOUT, sh, W], f16, tag="yout")
                nchunk = (sh + 2) // 3
                for c in range(nchunk):
                    rr0 = c * 3
                    nrr = min(3, sh - rr0)
                    ps = cv_psum.tile([NOUT, 3, WROW], f32, tag="cv")
                    nc.tensor.matmul(
                        ps[:, :nrr, :],
                        wb_t[:, :],
                        p9[:, rr0 * WROW : (rr0 + nrr) * WROW],
                        start=True,
                        stop=True,
                    )
                    if c % 2 == 0:
                        nc.vector.tensor_copy(
                            yt[:, rr0 : rr0 + nrr, :], ps[:, :nrr, 0:W]
                        )
                    else:
                        nc.scalar.copy(yt[:, rr0 : rr0 + nrr, :], ps[:, :nrr, 0:W])

                half = (sh // 2) if sh > 8 else sh
                nc.sync.dma_start(
                    out=y.ap()[:, r0 * W : (r0 + half) * W],
                    in_=yt[:, :half, :],
                )
                if half < sh:
                    nc.sync.dma_start(
                        out=y.ap()[:, (r0 + half) * W : (r0 + sh) * W],
                        in_=yt[:, half:, :],
                    )

            DEPTH = 2
            p9s = {}
            xins = {s: emit_in(s) for s in range(NS)}
            for s in range(NS + DEPTH):
                if s < NS:
                    p9s[s] = emit_cs_and_p(s, xins[s])
                emit_warm(6)
                if s >= DEPTH:
                    emit_cv_and_out(s - DEPTH, p9s[s - DEPTH])

    nc.compile()
    return nc


def _host_prep(x, weight, bias):
    fh = np.float16
    wsum = weight.sum(axis=1)  # [COUT, 3, 3] fp32
    wb = np.zeros((KCONV, NOUT), np.float32)
    for b in range(BPC):
        for u in range(3):
            for v in range(3):
                wb[u + 3 * v + 9 * b, b * COUT : (b + 1) * COUT] = wsum[:, u, v]
    wb[KCONV - 1, :] = np.tile(bias, BPC)
    wb = wb.astype(fh)
    ones_cs = np.zeros((NPART, BPC * 3), np.float32)
    for b in range(BPC):
        ones_cs[b * CIN : (b + 1) * CIN, b * 3 : (b + 1) * 3] = 1.0
    ones_cs = ones_cs.astype(fh)
    ones_p = np.ones((1, PWIN_MAX), dtype=fh)

    in_maps = []
    for r in range(N_CORES):
        xs = np.ascontiguousarray(
            x[r * BPC : (r + 1) * BPC].reshape(NPART, H, W)
        ).astype(fh)
        xpack = np.empty((NPART, XPACK_LEN), dtype=fh)
        for s in range(NS):
            h0, he = _SLICE_ROWS[s]
            n = (he - h0) * W
            o = _SLICE_OFF[s]
            xpack[:, o : o + n] = xs[:, h0:he].reshape(NPART, n)
        in_maps.append(
            {
                "xpack": xpack,
                "ones_cs": ones_cs,
                "wb": wb,
                "ones_p": ones_p,
            }
        )
    return in_maps


def kernel(x, weight, bias):
    from concourse.bass_utils import run_bass_kernel_spmd

    x = np.asarray(x)
    weight = np.asarray(weight)
    bias = np.asarray(bias)
    nc = _build()
    in_maps = _host_prep(x, weight, bias)
    res = run_bass_kernel_spmd(nc, in_maps, core_ids=list(range(N_CORES)))
    out = np.concatenate(
        [
            res.results[r]["y"].astype(np.float32).reshape(BPC, COUT, H, W)
            for r in range(N_CORES)
        ],
        axis=0,
    )
    return out
